# revision 16
# baseline (speedup 1.0000x reference)
"""Antialiased 2x upsampling (StyleGAN2 upsample_2d, k=[1,3,3,1], factor=2).

Input  x: (8, 256, 256, 64) f32 NHWC  ->  output: (8, 511, 511, 64) f32.

Math (separable, polyphase):
  g[i] = x[i-1]/3 + x[i]   (even out row 2i),  h[i] = x[i]/3 + x[i-1] (odd 2i-1)
  out[2i,   2j]   = 9/16*g[j]   + 3/16*g[j-1]
  out[2i,   2j-1] = 9/16*g[j-1] + 3/16*g[j]     (same for h on odd rows)

Sharding: pure data parallel, one batch image per NeuronCore (8 cores).

v3 design (TensorEngine does ALL the arithmetic):
- x is loaded ONCE per tile (128 rows incl. a 1-row halo) as bf16 (DMA casts
  in flight; HBM traffic stays f32). Removes the baseline's second
  (row-shifted) HBM read of x (~17MB/core).
- The H-pass is a banded [128->127] matmul (partition-shift done by the PE),
  and the W-pass col-shift is folded in via PSUM ACCUMULATION: for each
  512-f32 PSUM region, mm(W9, cols w) with start=True then
  mm(W3, cols w-1) with start=False accumulate the FINAL output value
  stream: Pe[n] = 9/16 c[w] + 3/16 c[w-1] (even out cols; c = g or h row
  combo), Po[n] = 3/16 c[w] + 9/16 c[w-1] (odd cols). W9[q,p] =
  3/16 d(q,p) + 9/16 d(q,p+1) (g block; h block swapped), W3 = W9/3.
  All weights ({9,3,1}/16) are exact in bf16.
- The only vector/scalar work left is PSUM -> SBUF interleave copies into
  the bf16 rowbuf: even-col streams on DVE, odd-col on ACT (both engines
  otherwise idle; ISA forbids two-PSUM-operand DVE ops anyway).
- PSUM: tags pe/po, [128, 1024] f32 (2 banks) x bufs=2 = all 8 banks;
  groups of 16 out-cols; the two row-parities ping-pong the rotation.
- WT=128 -> 64KB f32 store packets per partition-row (2x baseline), one
  descriptor ring entry each; loads are 33KB f32 -> 16.5KB bf16 packets.
- Edge out rows (0, 509, 510) are a 3-partition pass with a 3x3 weight
  block (x[254], x[255], x[0] stacked), scattered through the main loop
  in 4 w-quarters; its copies go to ACT only.
"""

import numpy as np

import concourse.bacc as bacc
import concourse.mybir as mybir
from concourse.tile import TileContext
from concourse.bass_utils import run_bass_kernel_spmd

F32 = mybir.dt.float32
BF16 = mybir.dt.bfloat16

B_FULL, H_FULL, W_FULL, C_FULL = 8, 256, 256, 64
N_CORES = 8


def make_weights():
    """[128, 514] f32: cols 0:257 = W9 (g 0:127 | h 127:254 | edge 254:257),
    cols 257:514 = W3 = W9/3 (exact: {9,3}/16 -> {3,1}/16)."""
    w9 = np.zeros((128, 257), dtype=np.float32)
    for p in range(127):
        # g9[p] = 3/16 x[i-1] + 9/16 x[i] = 3/16 B[p] + 9/16 B[p+1]
        w9[p, p] = 3.0 / 16.0
        w9[p + 1, p] = 9.0 / 16.0
        # h9[p] = 9/16 B[p] + 3/16 B[p+1]
        w9[p, 127 + p] = 9.0 / 16.0
        w9[p + 1, 127 + p] = 3.0 / 16.0
    # edge: partitions {x[254], x[255], x[0]} -> rows {509 (h@255), 510 (g@255), 0 (g@0)}
    w9[0, 254] = 9.0 / 16.0  # h9[255] = 9/16 x[254] + 3/16 x[255]
    w9[1, 254] = 3.0 / 16.0
    w9[0, 255] = 3.0 / 16.0  # g9[255] = 3/16 x[254] + 9/16 x[255]
    w9[1, 255] = 9.0 / 16.0
    w9[2, 256] = 9.0 / 16.0  # g9[0] = 9/16 x[0]   (x[-1] = 0)
    return np.concatenate([w9, w9 / 3.0], axis=1)


def build_upsample_tile(tc, out, x, w9d, H, W, C):
    nc = tc.nc
    WT = 128
    n_wt = W // WT
    FW = (WT + 1) * C          # 8256: halo col w0-1 plus WT cols
    seg = 2 * WT * C           # 16384: one output-row segment (2*WT out cols)
    PT = 127                   # out rows per h-tile (B tile holds PT+1 = 128 rows)
    n_ht = 2
    assert n_ht * PT == H - 2  # main tiles: i = 1..254 (out rows 1..508)
    # edge pass covers out rows 0, 509, 510

    GW = 16                    # out-cols per psum group (2 banks = 1024 f32)
    n_grp = WT // GW           # 8
    EQ = 4                     # edge pass split into 4 w-quarters of 64 cols
    EW = W // EQ               # 64
    eFW = (EW + 1) * C         # 4160
    eseg = 2 * EW * C          # 8192

    with (
        tc.tile_pool(name="io", bufs=2) as io_pool,
        tc.tile_pool(name="rb", bufs=2) as rb_pool,
        tc.tile_pool(name="ep", bufs=1) as ep_pool,
        tc.tile_pool(name="cst", bufs=1) as cst_pool,
        tc.tile_pool(name="ps", bufs=2, space="PSUM") as ps_pool,
    ):
        # ---- weights -> SBUF (bf16; all values exact)
        w9s = cst_pool.tile([128, 514], BF16, tag="w9", name="w9s")
        nc.gpsimd.dma_start(out=w9s[:], in_=w9d[:, :])

        def wg(ofs, n=PT):     # 9/16-band weight block
            return w9s[:, ofs : ofs + n]

        def w3(ofs, n=PT):     # 3/16-band weight block
            return w9s[:, 257 + ofs : 257 + ofs + n]

        def pchunks():
            return [(0, 64), (64, 127)]

        # ---------- main tiles ----------
        def load(s):
            t, wt = s // n_wt, s % n_wt
            r0 = 127 * t                     # B rows r0 .. r0+127
            Bt = io_pool.tile([128, FW], BF16, tag="B", name=f"B_{t}_{wt}")
            if wt == 0:
                nc.vector.memset(Bt[:, 0:C], 0.0)
                lo = C
            else:
                lo = 0
            cl = (wt * WT - 1) * C           # x col offset of tile col 0
            for q0, q1 in ((0, 64), (64, 128)):
                nc.gpsimd.dma_start(
                    out=Bt[q0:q1, lo:FW],
                    in_=x[r0 + q0 : r0 + q1, cl + lo : cl + FW],
                )
            return Bt

        def group_ops(Bt, rbv, u, PTl=PT, wofs_g=0, wofs_h=127):
            """One psum group: out-cols w0+16u .. w0+16u+15, both row parities."""
            base = u * GW
            for s_seg, wofs in ((1, wofs_g), (0, wofs_h)):
                Pe = ps_pool.tile([128, 1024], F32, tag="pe", name=f"pe{u}_{s_seg}")
                Po = ps_pool.tile([128, 1024], F32, tag="po", name=f"po{u}_{s_seg}")
                for o in (0, 512):
                    rw = Bt[:, (base + 1) * C + o : (base + 1) * C + o + 512]
                    rwm = Bt[:, base * C + o : base * C + o + 512]
                    nc.tensor.matmul(Pe[:PTl, o : o + 512], wg(wofs, PTl), rw,
                                     start=True, stop=False)
                    nc.tensor.matmul(Pe[:PTl, o : o + 512], w3(wofs, PTl), rwm,
                                     start=False, stop=True)
                    nc.tensor.matmul(Po[:PTl, o : o + 512], w3(wofs, PTl), rw,
                                     start=True, stop=False)
                    nc.tensor.matmul(Po[:PTl, o : o + 512], wg(wofs, PTl), rwm,
                                     start=False, stop=True)
                # interleave copies: even cols (q=1) on DVE, odd (q=0) on ACT
                nc.vector.tensor_copy(
                    out=rbv[:PTl, s_seg, base : base + GW, 1, :], in_=Pe[:PTl, :]
                )
                nc.scalar.copy(
                    out=rbv[:PTl, s_seg, base : base + GW, 0, :], in_=Po[:PTl, :]
                )

        def compute(s, Bt, edge_hook):
            t, wt = s // n_wt, s % n_wt
            rb = rb_pool.tile([128, 2 * seg], BF16, tag="rb", name=f"rb_{t}_{wt}")
            rbv = rb.rearrange("p (s j q c) -> p s j q c", s=2, j=WT, q=2, c=C)
            for u in range(n_grp):
                group_ops(Bt, rbv, u)
                if edge_hook is not None and u == 2:
                    edge_hook()
            return rb

        def store(s, rb):
            t, wt = s // n_wt, s % n_wt
            i0 = 1 + 127 * t
            skip = C if wt == 0 else 0
            dcol = 0 if wt == 0 else (2 * wt * WT - 1) * C
            dw = seg - skip
            for q0, q1 in pchunks():
                r0 = 2 * (i0 + q0) - 1
                nc.gpsimd.dma_start(
                    out=out[r0 : r0 + 2 * (q1 - q0) - 1 : 2, dcol : dcol + dw],
                    in_=rb[q0:q1, skip:seg],
                )
            for q0, q1 in pchunks():
                r0 = 2 * (i0 + q0)
                nc.gpsimd.dma_start(
                    out=out[r0 : r0 + 2 * (q1 - q0) - 1 : 2, dcol : dcol + dw],
                    in_=rb[q0:q1, seg + skip : 2 * seg],
                )

        # ---------- edge pass (out rows 509, 510, 0) in 4 w-quarters ----------
        def edge_load(wq):
            Be = ep_pool.tile([3, eFW], BF16, tag="Be", name=f"Be_{wq}")
            if wq == 0:
                nc.vector.memset(Be[:, 0:C], 0.0)
                lo = C
            else:
                lo = 0
            cl = (wq * EW - 1) * C
            nc.gpsimd.dma_start(out=Be[0:2, lo:eFW], in_=x[254:256, cl + lo : cl + eFW])
            nc.gpsimd.dma_start(out=Be[2:3, lo:eFW], in_=x[0:1, cl + lo : cl + eFW])
            return Be

        def edge_compute(wq, Be):
            rbe = ep_pool.tile([3, eseg], BF16, tag="rbe", name=f"rbe_{wq}")
            rbev = rbe.rearrange("p (j q c) -> p j q c", j=EW, q=2, c=C)
            for v in range(EW // GW):
                base = v * GW
                Pe = ps_pool.tile([128, 1024], F32, tag="pe", name=f"epe{wq}_{v}")
                Po = ps_pool.tile([128, 1024], F32, tag="po", name=f"epo{wq}_{v}")
                for o in (0, 512):
                    rw = Be[:3, (base + 1) * C + o : (base + 1) * C + o + 512]
                    rwm = Be[:3, base * C + o : base * C + o + 512]
                    nc.tensor.matmul(Pe[:3, o : o + 512], w9s[0:3, 254:257], rw,
                                     start=True, stop=False)
                    nc.tensor.matmul(Pe[:3, o : o + 512], w9s[0:3, 511:514], rwm,
                                     start=False, stop=True)
                    nc.tensor.matmul(Po[:3, o : o + 512], w9s[0:3, 511:514], rw,
                                     start=True, stop=False)
                    nc.tensor.matmul(Po[:3, o : o + 512], w9s[0:3, 254:257], rwm,
                                     start=False, stop=True)
                nc.scalar.copy(out=rbev[:3, base : base + GW, 1, :], in_=Pe[:3, :])
                nc.scalar.copy(out=rbev[:3, base : base + GW, 0, :], in_=Po[:3, :])
            return rbe

        def edge_store(wq, rbe):
            skip = C if wq == 0 else 0
            dcol = 0 if wq == 0 else (2 * wq * EW - 1) * C
            dw = eseg - skip
            nc.gpsimd.dma_start(
                out=out[509:511, dcol : dcol + dw], in_=rbe[0:2, skip:eseg]
            )
            nc.gpsimd.dma_start(
                out=out[0:1, dcol : dcol + dw], in_=rbe[2:3, skip:eseg]
            )

        # ---------- pipeline ----------
        N = n_ht * n_wt                      # 4 main steps
        PRE = 2
        btiles = {}
        for s in range(min(PRE, N)):
            btiles[s] = load(s)
        ebuf = {"B": edge_load(0), "rb": None}

        def edge_hook_step(s):
            if s >= EQ:
                return None

            def hook():
                wq = s
                rbe = edge_compute(wq, ebuf["B"])
                ebuf["rb"] = rbe
                if wq + 1 < EQ:
                    ebuf["B"] = edge_load(wq + 1)
            return hook

        for s in range(N):
            if s + PRE < N:
                btiles[s + PRE] = load(s + PRE)
            rb = compute(s, btiles.pop(s), edge_hook_step(s))
            store(s, rb)
            if ebuf["rb"] is not None:
                edge_store(s, ebuf["rb"])
                ebuf["rb"] = None


def build_nc(H=H_FULL, W=W_FULL, C=C_FULL):
    nc = bacc.Bacc(
        "TRN2", target_bir_lowering=False, debug=False,
        dynamic_dma_scratch_size=16384,
    )
    x = nc.declare_dram_parameter("x", [H, W * C], F32, isOutput=False).ap()
    w9d = nc.declare_dram_parameter("w9", [128, 514], F32, isOutput=False).ap()
    out = nc.declare_dram_parameter(
        "out", [2 * H - 1, (2 * W - 1) * C], F32, isOutput=True
    ).ap()
    with TileContext(nc) as tc:
        build_upsample_tile(tc, out, x, w9d, H, W, C)
    nc.compile()
    return nc


_NC_CACHE = {}


def _get_nc():
    key = (H_FULL, W_FULL, C_FULL)
    if key not in _NC_CACHE:
        _NC_CACHE[key] = build_nc()
    return _NC_CACHE[key]


def run_spmd(x, trace=False, **kwargs):
    """x: (8, 256, 256, 64) f32. Returns (BassKernelResults, out (8,511,511,64))."""
    nc = _get_nc()
    w9 = make_weights()
    in_maps = [
        {
            "x": np.ascontiguousarray(x[b]).reshape(H_FULL, W_FULL * C_FULL),
            "w9": w9,
        }
        for b in range(N_CORES)
    ]
    res = run_bass_kernel_spmd(
        nc, in_maps, core_ids=list(range(N_CORES)), trace=trace, **kwargs
    )
    out = np.stack(
        [
            res.results[b]["out"].reshape(2 * H_FULL - 1, 2 * W_FULL - 1, C_FULL)
            for b in range(N_CORES)
        ]
    )
    return res, out


def kernel(x):
    x = np.asarray(x, dtype=np.float32)
    _, out = run_spmd(x, trace=False)
    return out
